# revision 15
# baseline (speedup 1.0000x reference)
# Trainium2 Bass kernel for the DisentangledCodebooks problem.
#
# Three independent VQ streams (t/g/e). Per stream:
#   h  = relu(x @ W1 + b1)                    (B, 256)
#   z  = h @ W2 + b2                          (B, N*256) -> (B, N, 256)
#   idx_n = argmin_k ||z_n - cb_k||^2         per code slot n
#   q  = cb[idx]                              (straight-through value == q)
#   out = relu(q_flat @ dW1 + db1) @ dW2 + db2
#   loss = 0.25 * mean((z - q)^2)
#
# Sharding: data parallel over batch, 2048 rows per core on 8 cores; all
# weights/codebooks replicated.
#
# Device mapping highlights:
#  - activations kept transposed (features on partitions, batch on free dim)
#    so chained matmuls need no transposes; x is transposed on-chip via PE.
#  - distance argmin: PE computes p = z @ (2*cb^T); DVE subtracts |cb|^2
#    (nd == |z|^2 - d up to a per-row constant), reduces the row max, and
#    max_index yields argmin with first-occurrence tie-break (== jnp.argmin).
#  - decoder first layer: pre = sum_n G_n[idx_n] where G_n = 2*cb @ dW1_n
#    (+ 2*db1 folded into G_0) is precomputed on device into one (N*K, D)
#    DRAM table; one dma_gather per 128-row tile fetches all N rows/sample
#    (indices bounced through DRAM into the 16-partition wrap layout), then
#    relu(0.5 * pre) == relu(q @ dW1 + db1) exactly.
#  - loss uses sum(d_min) = sum|z|^2 + sum(min_k(|c|^2 - 2 s)); partial sums
#    per partition are written out and reduced on host.
#  - per stream the work is split into phase 1 (encode + VQ + index staging)
#    and phase 2 (gather + decode); phase-2 chunks are emitted two chunks
#    behind the NEXT stream's phase 1 so every in-order engine always has
#    ready work. DMA issue is split by dependency shape: SP carries loads
#    whose inputs are ready at issue (weights, x, codebooks, idx reloads),
#    ACT stores data it just produced (G tables, outputs), GpSimd carries
#    the index stores/gathers that wait on the VQ tail.

from contextlib import ExitStack

import numpy as np

D = 256
B = 16384
NCORES = 8
BLOC = B // NCORES
BETA = 0.25
STREAMS = [("t", 3, 500), ("g", 4, 1000), ("e", 3, 1000)]

_BUILT = {}


def build(rows=BLOC, debug=False, enable_asserts=False):
    import concourse.bass as bass
    import concourse.mybir as mybir
    import concourse.tile as tile
    from concourse import bacc
    from concourse.masks import make_identity
    from concourse.tile_rust import add_dep_helper

    fp32 = mybir.dt.float32
    i32 = mybir.dt.int32
    i16 = mybir.dt.int16
    u16 = mybir.dt.uint16
    Alu = mybir.AluOpType
    Act = mybir.ActivationFunctionType
    NEG = -3.0e38

    CH = min(512, rows)          # batch chunk through the encoder
    assert rows % CH == 0
    NCH = rows // CH
    RT = CH // 128               # row tiles per chunk
    assert CH % 128 == 0

    nc = bacc.Bacc(
        "TRN2",
        target_bir_lowering=False,
        debug=debug,
        enable_asserts=enable_asserts,
        num_devices=NCORES,
    )

    # ---------------- DRAM I/O ----------------
    x_in = {}
    wts = {}
    for p, N, K in STREAMS:
        x_in[p] = nc.dram_tensor(f"x_{p}", (rows, D), fp32, kind="ExternalInput")
        for wname, shape in (
            ("W1", (D, D)), ("b1", (D,)),
            ("W2", (D, N * D)), ("b2", (N * D,)),
            ("cb", (K, D)),
            ("dW1", (N * D, D)), ("db1", (D,)),
            ("dW2", (D, D)), ("db2", (D,)),
        ):
            wts[(p, wname)] = nc.dram_tensor(
                f"{p}_{wname}", shape, fp32, kind="ExternalInput"
            )

    out_d = nc.dram_tensor("out", (rows, 3, D), fp32, kind="ExternalOutput")
    # staged (idx + n*K) int16 values double as the index output; the host
    # subtracts the n*K offsets
    idx_d = {
        p: nc.dram_tensor(
            f"idxs_{p}", (rows // CH, (CH // 128) * N * 128), i16,
            kind="ExternalOutput",
        )
        for p, N, K in STREAMS
    }
    lossm_d = nc.dram_tensor("loss_m", (3, 128), fp32, kind="ExternalOutput")
    lossz_d = nc.dram_tensor("loss_z", (3, 128), fp32, kind="ExternalOutput")

    with tile.TileContext(nc) as tc, ExitStack() as ctx:
        consts = ctx.enter_context(tc.tile_pool(name="consts", bufs=1))
        swp = ctx.enter_context(tc.tile_pool(name="swp", bufs=2))    # stream-lived
        gwk = ctx.enter_context(tc.tile_pool(name="gwk", bufs=2))    # G' staging
        chk = ctx.enter_context(tc.tile_pool(name="chk", bufs=2))    # chunk-lived
        vqp = ctx.enter_context(tc.tile_pool(name="vqp", bufs=3))
        dec = ctx.enter_context(tc.tile_pool(name="dec", bufs=3))
        acc = ctx.enter_context(tc.tile_pool(name="acc", bufs=2))
        ptr = ctx.enter_context(tc.tile_pool(name="ptr", bufs=2, space="PSUM"))
        pmm = ctx.enter_context(tc.tile_pool(name="pmm", bufs=2, space="PSUM"))
        gdp = ctx.enter_context(tc.tile_pool(name="gdp", bufs=1, space="DRAM"))

        ident = consts.tile([128, 128], fp32)
        make_identity(nc, ident)
        ones_row = consts.tile([1, 128], fp32)
        nc.vector.memset(ones_row, 1.0)
        ones_col = consts.tile([128, 1], fp32)
        nc.vector.memset(ones_col, 1.0)

        S = [dict() for _ in STREAMS]  # cross-phase per-stream state

        def p1setup(si):
            p, N, K = STREAMS[si]
            KT = (K + 127) // 128

            # weights (SP: no dependencies at issue)
            W1t = swp.tile([128, 2, D], fp32, tag="W1t")
            nc.sync.dma_start(
                out=W1t, in_=wts[(p, "W1")][:].rearrange("(a k) m -> k a m", k=128)
            )
            W2t = swp.tile([128, 2, N * D], fp32, tag="W2t")
            nc.sync.dma_start(
                out=W2t, in_=wts[(p, "W2")][:].rearrange("(a k) m -> k a m", k=128)
            )
            dW2t = swp.tile([128, 2, D], fp32, tag="dW2t")
            nc.sync.dma_start(
                out=dW2t, in_=wts[(p, "dW2")][:].rearrange("(a k) m -> k a m", k=128)
            )
            b1c = swp.tile([128, 2], fp32, tag="b1c")
            nc.sync.dma_start(
                out=b1c, in_=wts[(p, "b1")][:].rearrange("(m k) -> k m", k=128)
            )
            b2c = swp.tile([128, 2 * N], fp32, tag="b2c")
            nc.sync.dma_start(
                out=b2c, in_=wts[(p, "b2")][:].rearrange("(m k) -> k m", k=128)
            )
            db1f2 = swp.tile([128, D], fp32, tag="db1f2")
            nc.sync.dma_start(
                out=db1f2,
                in_=wts[(p, "db1")][:]
                .rearrange("(a d) -> a d", a=1)
                .to_broadcast([128, D]),
            )
            nc.scalar.activation(out=db1f2, in_=db1f2, func=Act.Copy, scale=2.0)
            db2f = swp.tile([128, D], fp32, tag="db2f")
            nc.sync.dma_start(
                out=db2f,
                in_=wts[(p, "db2")][:]
                .rearrange("(a d) -> a d", a=1)
                .to_broadcast([128, D]),
            )

            # prefetch all x chunks up front
            x4s = []
            for c in range(NCH):
                x4 = chk.tile([128, RT, D], fp32, tag="x4", bufs=NCH)
                nc.sync.dma_start(
                    out=x4,
                    in_=x_in[p][c * CH : (c + 1) * CH, :].rearrange(
                        "(j q) d -> q j d", q=128
                    ),
                )
                x4s.append(x4)

            # transpose codebook: cbT2[:, h*K + j] = 2 * cb[j, h*128 + p]
            cbT2 = swp.tile([128, 2 * K], fp32, tag="cbT2")
            for kt in range(KT):
                ksz = min(128, K - kt * 128)
                cbrow = swp.tile([128, D], fp32, tag="cbrow", bufs=3)
                nc.sync.dma_start(
                    out=cbrow[:ksz], in_=wts[(p, "cb")][kt * 128 : kt * 128 + ksz, :]
                )
                for h in range(2):
                    tps = ptr.tile([128, 128], fp32, tag="tr")
                    nc.tensor.transpose(
                        out=tps[:, :ksz],
                        in_=cbrow[:ksz, h * 128 : (h + 1) * 128],
                        identity=ident[:ksz, :ksz],
                    )
                    nc.scalar.activation(
                        out=cbT2[:, h * K + kt * 128 : h * K + kt * 128 + ksz],
                        in_=tps[:, :ksz],
                        func=Act.Copy,
                        scale=2.0,
                    )

            # c2 = sum_d cb[k,d]^2, materialized across partitions
            sqT = swp.tile([128, 2 * K], fp32, tag="sqT")
            nc.vector.tensor_tensor(out=sqT, in0=cbT2, in1=cbT2, op=Alu.mult)
            c2s = swp.tile([1, K], fp32, tag="c2s")
            for cb_ofs in range(0, K, 512):
                csz = min(512, K - cb_ofs)
                pc2 = pmm.tile([1, 512], fp32, tag="dist", bufs=3)
                for h in range(2):
                    nc.tensor.matmul(
                        out=pc2[:, :csz],
                        lhsT=ones_col,
                        rhs=sqT[:, h * K + cb_ofs : h * K + cb_ofs + csz],
                        start=(h == 0),
                        stop=(h == 1),
                    )
                nc.scalar.activation(
                    out=c2s[:, cb_ofs : cb_ofs + csz],
                    in_=pc2[:, :csz],
                    func=Act.Copy,
                    scale=0.25,
                )
            c2d = gdp.tile([1, K], fp32, tag=f"c2d_{p}", name=f"c2d_{p}")
            nc.scalar.dma_start(out=c2d, in_=c2s)
            c2full = swp.tile([128, K], fp32, tag="c2full")
            nc.scalar.dma_start(out=c2full, in_=c2d[:].to_broadcast([128, K]))

            # G'_n = 2*cb @ dW1_n (+ 2*db1 folded into G'_0), concatenated
            # into one (N*K, D) table so one dma_gather serves all N codes
            gcat = gdp.tile([N * K, D], fp32, tag=f"gcat_{p}", name=f"gcat_{p}")
            stg = idx_d[p]
            offc = swp.tile([128, N], i16, tag="offc")
            for n in range(N):
                nc.vector.memset(offc[:, n : n + 1], n * K)
            for n in range(N):
                dW1n = gwk.tile([128, 2, D], fp32, tag="dW1n")
                nc.sync.dma_start(
                    out=dW1n,
                    in_=wts[(p, "dW1")][n * D : (n + 1) * D, :].rearrange(
                        "(a k) m -> k a m", k=128
                    ),
                )
                for mt in range(KT):
                    msz = min(128, K - mt * 128)
                    pg = pmm.tile([128, D], fp32, tag="dist", bufs=3)
                    for h in range(2):
                        nc.tensor.matmul(
                            out=pg[:msz],
                            lhsT=cbT2[:, h * K + mt * 128 : h * K + mt * 128 + msz],
                            rhs=dW1n[:, h, :],
                            start=(h == 0),
                            stop=(h == 1),
                        )
                    gsb = gwk.tile([128, D], fp32, tag="gsb", bufs=3)
                    if n == 0:
                        # every row of table 0 also gets +2*db1
                        nc.vector.tensor_tensor(
                            out=gsb[:msz], in0=pg[:msz], in1=db1f2[:msz],
                            op=Alu.add,
                        )
                    else:
                        nc.scalar.activation(
                            out=gsb[:msz], in_=pg[:msz], func=Act.Copy
                        )
                    nc.scalar.dma_start(
                        out=gcat[n * K + mt * 128 : n * K + mt * 128 + msz, :],
                        in_=gsb[:msz],
                    )

            # per-stream accumulators
            macc = acc.tile([128, 1], fp32, tag="macc")
            nc.vector.memset(macc, 0.0)
            zacc = acc.tile([128, 1], fp32, tag="zacc")
            nc.vector.memset(zacc, 0.0)

            S[si].update(
                W1t=W1t, W2t=W2t, dW2t=dW2t, b1c=b1c, b2c=b2c, db2f=db2f,
                cbT2=cbT2, c2full=c2full, gcat=gcat, stg=stg, offc=offc,
                macc=macc, zacc=zacc, x4s=x4s,
            )

        def p1chunk(si, c):
            p, N, K = STREAMS[si]
            KB = (K + 511) // 512
            st = S[si]
            x4 = st["x4s"][c]

            xT = chk.tile([128, 2 * CH], fp32, tag="xT")
            for j in range(RT):
                for h in range(2):
                    tps = ptr.tile([128, 128], fp32, tag="tr")
                    nc.tensor.transpose(
                        out=tps, in_=x4[:, j, h * 128 : (h + 1) * 128],
                        identity=ident,
                    )
                    nc.scalar.activation(
                        out=xT[:, h * CH + j * 128 : h * CH + (j + 1) * 128],
                        in_=tps,
                        func=Act.Copy,
                    )

            # encoder layer 1: h1T = relu(W1-slices^T @ xT + b1)
            h1T = chk.tile([128, 2 * CH], fp32, tag="h1T")
            for m in range(2):
                ph = pmm.tile([128, CH], fp32, tag="enc", bufs=2)
                for h in range(2):
                    nc.tensor.matmul(
                        out=ph,
                        lhsT=st["W1t"][:, h, m * 128 : (m + 1) * 128],
                        rhs=xT[:, h * CH : (h + 1) * CH],
                        start=(h == 0),
                        stop=(h == 1),
                    )
                nc.scalar.activation(
                    out=h1T[:, m * CH : (m + 1) * CH],
                    in_=ph,
                    func=Act.Relu,
                    bias=st["b1c"][:, m : m + 1],
                )

            # encoder layer 2: zT = W2-slices^T @ h1T + b2
            zT = chk.tile([128, 2 * N * CH], fp32, tag="zT")
            for m in range(2 * N):
                pz = pmm.tile([128, CH], fp32, tag="enc", bufs=2)
                for h in range(2):
                    nc.tensor.matmul(
                        out=pz,
                        lhsT=st["W2t"][:, h, m * 128 : (m + 1) * 128],
                        rhs=h1T[:, h * CH : (h + 1) * CH],
                        start=(h == 0),
                        stop=(h == 1),
                    )
                nc.scalar.activation(
                    out=zT[:, m * CH : (m + 1) * CH],
                    in_=pz,
                    func=Act.Identity,
                    bias=st["b2c"][:, m : m + 1],
                )
                # |z|^2 partial sums (for the loss)
                zsq = acc.tile([128, CH], fp32, tag="zsq")
                z2t = acc.tile([128, 1], fp32, tag="z2t", bufs=4)
                nc.scalar.activation(
                    out=zsq,
                    in_=zT[:, m * CH : (m + 1) * CH],
                    func=Act.Square,
                    accum_out=z2t,
                )
                nc.vector.tensor_tensor(
                    out=st["zacc"], in0=st["zacc"], in1=z2t, op=Alu.add
                )

            # VQ per 128-row tile
            last_mm = None
            last_store = None
            for r in range(RT):
                rt = c * RT + r
                idx16c = vqp.tile([128, N], i16, tag="idx16c")
                for n in range(N):
                    nd = vqp.tile([128, K], fp32, tag="nd")
                    maxv = vqp.tile([128, 8], fp32, tag="maxv", bufs=4)
                    nc.vector.memset(maxv, NEG)
                    for b in range(KB):
                        ksz = min(512, K - b * 512)
                        pd = pmm.tile([128, 512], fp32, tag="dist", bufs=3)
                        for h in range(2):
                            last_mm = nc.tensor.matmul(
                                out=pd[:, :ksz],
                                lhsT=zT[
                                    :,
                                    (2 * n + h) * CH
                                    + r * 128 : (2 * n + h) * CH
                                    + (r + 1) * 128,
                                ],
                                rhs=st["cbT2"][
                                    :, h * K + b * 512 : h * K + b * 512 + ksz
                                ],
                                start=(h == 0),
                                stop=(h == 1),
                            )
                        # nd = pd - c2   (== |z|^2 - d, up to a row const)
                        nc.vector.tensor_tensor(
                            out=nd[:, b * 512 : b * 512 + ksz],
                            in0=pd[:, :ksz],
                            in1=st["c2full"][:, b * 512 : b * 512 + ksz],
                            op=Alu.subtract,
                        )
                    nc.vector.tensor_reduce(
                        out=maxv[:, 0:1], in_=nd, axis=mybir.AxisListType.X,
                        op=Alu.max,
                    )
                    idx8 = vqp.tile([128, 8], u16, tag="idx8", bufs=4)
                    nc.vector.max_index(out=idx8, in_max=maxv, in_values=nd)
                    # loss: macc -= max  (sum of min-dist partials)
                    nc.vector.tensor_tensor(
                        out=st["macc"], in0=st["macc"], in1=maxv[:, 0:1],
                        op=Alu.subtract,
                    )
                    # idx + n*K (int16): gather offsets AND index output
                    nc.vector.tensor_tensor(
                        out=idx16c[:, n : n + 1],
                        in0=idx8[:, 0:1].bitcast(i16),
                        in1=st["offc"][:, n : n + 1],
                        op=Alu.add,
                    )

                # stage idx16 to DRAM; chunk layout: m = r*(N*128)+n*128+p
                # (GpSimd: waits on the VQ tail, nothing urgent behind it)
                last_store = nc.gpsimd.dma_start(
                    out=bass.AP(
                        tensor=st["stg"],
                        offset=(c * RT + r) * N * 128,
                        ap=[[1, 128], [128, N]],
                    ),
                    in_=idx16c,
                )

            st.setdefault("anchor_mm", {})[c] = last_mm
            st.setdefault("anchor_store", {})[c] = last_store

        def p1epilogue(si):
            p, N, K = STREAMS[si]
            st = S[si]
            nc.gpsimd.dma_start(
                out=lossm_d[si, :].rearrange("(a q) -> q a", a=1), in_=st["macc"]
            )
            nc.gpsimd.dma_start(
                out=lossz_d[si, :].rearrange("(a q) -> q a", a=1), in_=st["zacc"]
            )

        def p2chunk(si, c, anchor_mm=None, anchor_store=None):
            p, N, K = STREAMS[si]
            st = S[si]
            # reload idx16 in the 16-partition wrap layout, replicated across
            # the 8 partition groups (each GpSimd core reads its own):
            # idxs[g*16+q, r, n, j] = stg[c, r*N*128 + n*128 + j*16 + q]
            idx16w = dec.tile([128, RT, N, 8], i16, tag="idx16w", bufs=2)
            for g in range(8):
                nc.sync.dma_start(
                    out=idx16w[g * 16 : (g + 1) * 16, :, :, :],
                    in_=bass.AP(
                        tensor=st["stg"],
                        offset=c * (RT * N * 128),
                        ap=[[1, 16], [N * 128, RT], [128, N], [16, 8]],
                    ),
                )

            for r in range(RT):
                rt = c * RT + r
                wide = dec.tile([128, N, D], fp32, tag="wide")
                gthr = nc.gpsimd.dma_gather(
                    out_ap=wide,
                    in_ap=st["gcat"][:],
                    idxs_ap=idx16w[:, r, :, :].rearrange("q n j -> q (n j)"),
                    num_idxs=N * 128,
                    num_idxs_reg=N * 128,
                    elem_size=D,
                )
                if anchor_store is not None:
                    add_dep_helper(
                        gthr.ins, anchor_store.ins, sync=False,
                        reason="lag decode gathers behind the pipelined VQ",
                    )
                pre = dec.tile([128, D], fp32, tag="pre")
                nc.vector.tensor_tensor(
                    out=pre, in0=wide[:, 0, :], in1=wide[:, 1, :], op=Alu.add
                )
                for n in range(2, N):
                    nc.vector.tensor_tensor(
                        out=pre, in0=pre, in1=wide[:, n, :], op=Alu.add
                    )

                # decoder: h1d = relu(0.5*pre) ; out = h1dT-mm dW2 + db2
                h1d = dec.tile([128, D], fp32, tag="h1d")
                nc.scalar.activation(out=h1d, in_=pre, func=Act.Relu, scale=0.5)
                h1dT = dec.tile([128, 2 * 128], fp32, tag="h1dT")
                for h in range(2):
                    tps = ptr.tile([128, 128], fp32, tag="tr")
                    ti = nc.tensor.transpose(
                        out=tps, in_=h1d[:, h * 128 : (h + 1) * 128], identity=ident
                    )
                    if anchor_mm is not None:
                        add_dep_helper(
                            ti.ins, anchor_mm.ins, sync=False,
                            reason="keep decode PE work behind this chunk's VQ",
                        )
                    nc.scalar.activation(
                        out=h1dT[:, h * 128 : (h + 1) * 128], in_=tps, func=Act.Copy
                    )
                po = pmm.tile([128, D], fp32, tag="po", bufs=1)
                for h in range(2):
                    nc.tensor.matmul(
                        out=po,
                        lhsT=h1dT[:, h * 128 : (h + 1) * 128],
                        rhs=st["dW2t"][:, h, :],
                        start=(h == 0),
                        stop=(h == 1),
                    )
                outt = dec.tile([128, D], fp32, tag="outt")
                nc.vector.tensor_tensor(
                    out=outt, in0=po, in1=st["db2f"], op=Alu.add
                )
                nc.scalar.dma_start(
                    out=out_d[rt * 128 : (rt + 1) * 128, si, :], in_=outt
                )

        # ---- software pipeline: p2 runs two chunks behind the next
        # stream's p1 ----
        LAG = 2
        p1setup(0)
        for c in range(NCH):
            p1chunk(0, c)
        p1epilogue(0)
        p1setup(1)
        for c in range(NCH):
            p1chunk(1, c)
            if c >= LAG:
                p2chunk(
                    0, c - LAG,
                    anchor_mm=S[1]["anchor_mm"][c],
                    anchor_store=S[1]["anchor_store"][c - 1],
                )
        p1epilogue(1)
        for c in range(max(0, NCH - LAG), NCH):
            p2chunk(0, c)
        p1setup(2)
        for c in range(NCH):
            p1chunk(2, c)
            if c >= LAG:
                p2chunk(
                    1, c - LAG,
                    anchor_mm=S[2]["anchor_mm"][c],
                    anchor_store=S[2]["anchor_store"][c - 1],
                )
                p2chunk(
                    2, c - LAG,
                    anchor_mm=S[2]["anchor_mm"][c],
                    anchor_store=S[2]["anchor_store"][c - 1],
                )
        p1epilogue(2)
        for c in range(max(0, NCH - LAG), NCH):
            p2chunk(1, c)
        for c in range(max(0, NCH - LAG), NCH):
            p2chunk(2, c)

    nc.compile()
    return nc


def _get_nc(rows=BLOC, debug=False):
    key = (rows, debug)
    if key not in _BUILT:
        _BUILT[key] = build(rows, debug=debug)
    return _BUILT[key]


def make_in_map(inputs, core, rows=BLOC):
    sl = slice(core * rows, (core + 1) * rows)
    m = {
        "x_t": np.ascontiguousarray(np.asarray(inputs["sketch_features"])[sl]),
        "x_g": np.ascontiguousarray(np.asarray(inputs["geometry_features"])[sl]),
        "x_e": np.ascontiguousarray(np.asarray(inputs["extrusion_features"])[sl]),
    }
    for p, N, K in STREAMS:
        for w in ("W1", "b1", "W2", "b2", "cb", "dW1", "db1", "dW2", "db2"):
            m[f"{p}_{w}"] = np.ascontiguousarray(
                np.asarray(inputs[f"{p}_{w}"], dtype=np.float32)
            )
    return m


def assemble(results):
    """results: list of 8 per-core output dicts -> reference-shaped outputs."""
    out = np.concatenate([r["out"] for r in results], axis=0)
    idxs = {}
    for p, N, K in STREAMS:
        percore = []
        for r in results:
            a = r[f"idxs_{p}"].astype(np.int32)  # (NCH, RT*N*128), idx + n*K
            nch = a.shape[0]
            a = a.reshape(nch, -1, N, 128) - (np.arange(N, dtype=np.int32) * K)[
                None, None, :, None
            ]
            # value[c, r, n, q] is code for row c*CH + r*128 + q, slot n
            percore.append(np.ascontiguousarray(a.transpose(0, 1, 3, 2)).reshape(-1))
        idxs[p] = np.concatenate(percore, axis=0)
    loss = np.float32(0.0)
    for si, (p, N, K) in enumerate(STREAMS):
        tot = np.float32(0.0)
        for r in results:
            tot += r["loss_m"][si].sum(dtype=np.float32) + r["loss_z"][si].sum(
                dtype=np.float32
            )
        loss = np.float32(loss + np.float32(BETA) * tot / np.float32(B * N * D))
    return out, loss, idxs["t"], idxs["g"], idxs["e"]


def kernel(**inputs):
    from concourse import bass_utils

    nc = _get_nc()
    in_maps = [make_in_map(inputs, c) for c in range(NCORES)]
    res = bass_utils.run_bass_kernel_spmd(nc, in_maps, core_ids=list(range(NCORES)))
    return assemble(res.results)


# revision 16
# speedup vs baseline: 1.1014x; 1.1014x over previous
# Trainium2 Bass kernel for the DisentangledCodebooks problem.
#
# Three independent VQ streams (t/g/e). Per stream:
#   h  = relu(x @ W1 + b1)                    (B, 256)
#   z  = h @ W2 + b2                          (B, N*256) -> (B, N, 256)
#   idx_n = argmin_k ||z_n - cb_k||^2         per code slot n
#   q  = cb[idx]                              (straight-through value == q)
#   out = relu(q_flat @ dW1 + db1) @ dW2 + db2
#   loss = 0.25 * mean((z - q)^2)
#
# Sharding: data parallel over batch, 2048 rows per core on 8 cores; all
# weights/codebooks replicated.
#
# Device mapping highlights:
#  - activations kept transposed (features on partitions, batch on free dim)
#    so chained matmuls need no transposes; x is transposed on-chip via PE.
#  - distance argmin: PE computes p = z @ (2*cb^T); DVE subtracts |cb|^2
#    (nd == |z|^2 - d up to a per-row constant), reduces the row max, and
#    max_index yields argmin with first-occurrence tie-break (== jnp.argmin).
#  - decoder first layer: pre = sum_n G_n[idx_n] where G_n = 2*cb @ dW1_n
#    (+ 2*db1 folded into G_0) is precomputed on device into one (N*K, D)
#    DRAM table; one dma_gather per 128-row tile fetches all N rows/sample
#    (indices bounced through DRAM into the 16-partition wrap layout), then
#    relu(0.5 * pre) == relu(q @ dW1 + db1) exactly.
#  - loss uses sum(d_min) = sum|z|^2 + sum(min_k(|c|^2 - 2 s)); partial sums
#    per partition are written out and reduced on host.
#  - per stream the work is split into phase 1 (encode + VQ + index staging)
#    and phase 2 (gather + decode); phase-2 chunks are emitted two chunks
#    behind the NEXT stream's phase 1 so every in-order engine always has
#    ready work. DMA issue is split by dependency shape: SP carries loads
#    whose inputs are ready at issue (weights, x, codebooks, idx reloads),
#    ACT stores data it just produced (G tables, outputs), GpSimd carries
#    the index stores/gathers that wait on the VQ tail.

from contextlib import ExitStack

import numpy as np

D = 256
B = 16384
NCORES = 8
BLOC = B // NCORES
BETA = 0.25
STREAMS = [("t", 3, 500), ("g", 4, 1000), ("e", 3, 1000)]

_BUILT = {}


def build(rows=BLOC, debug=False, enable_asserts=False):
    import concourse.bass as bass
    import concourse.mybir as mybir
    import concourse.tile as tile
    from concourse import bacc
    from concourse.masks import make_identity
    from concourse.tile_rust import add_dep_helper

    fp32 = mybir.dt.float32
    i32 = mybir.dt.int32
    i16 = mybir.dt.int16
    u16 = mybir.dt.uint16
    Alu = mybir.AluOpType
    Act = mybir.ActivationFunctionType
    NEG = -3.0e38

    CH = min(512, rows)          # batch chunk through the encoder
    assert rows % CH == 0
    NCH = rows // CH
    RT = CH // 128               # row tiles per chunk
    assert CH % 128 == 0

    nc = bacc.Bacc(
        "TRN2",
        target_bir_lowering=False,
        debug=debug,
        enable_asserts=enable_asserts,
        num_devices=NCORES,
    )

    # ---------------- DRAM I/O ----------------
    x_in = {}
    wts = {}
    for p, N, K in STREAMS:
        x_in[p] = nc.dram_tensor(f"x_{p}", (rows, D), fp32, kind="ExternalInput")
        for wname, shape in (
            ("W1", (D, D)), ("b1", (D,)),
            ("W2", (D, N * D)), ("b2", (N * D,)),
            ("cb", (K, D)),
            ("dW1", (N * D, D)), ("db1", (D,)),
            ("dW2", (D, D)), ("db2", (D,)),
        ):
            wts[(p, wname)] = nc.dram_tensor(
                f"{p}_{wname}", shape, fp32, kind="ExternalInput"
            )

    out_d = nc.dram_tensor("out", (rows, 3, D), fp32, kind="ExternalOutput")
    # staged (idx + n*K) int16 values double as the index output; the host
    # subtracts the n*K offsets
    idx_d = {
        p: nc.dram_tensor(
            f"idxs_{p}", (rows // CH, (CH // 128) * N * 128), i16,
            kind="ExternalOutput",
        )
        for p, N, K in STREAMS
    }
    lossm_d = nc.dram_tensor("loss_m", (3, 128), fp32, kind="ExternalOutput")
    lossz_d = nc.dram_tensor("loss_z", (3, 128), fp32, kind="ExternalOutput")

    with tile.TileContext(nc) as tc, ExitStack() as ctx:
        consts = ctx.enter_context(tc.tile_pool(name="consts", bufs=1))
        swp = ctx.enter_context(tc.tile_pool(name="swp", bufs=2))    # stream-lived
        gwk = ctx.enter_context(tc.tile_pool(name="gwk", bufs=2))    # G' staging
        chk = ctx.enter_context(tc.tile_pool(name="chk", bufs=2))    # chunk-lived
        vqp = ctx.enter_context(tc.tile_pool(name="vqp", bufs=3))
        dec = ctx.enter_context(tc.tile_pool(name="dec", bufs=3))
        acc = ctx.enter_context(tc.tile_pool(name="acc", bufs=2))
        ptr = ctx.enter_context(tc.tile_pool(name="ptr", bufs=2, space="PSUM"))
        pmm = ctx.enter_context(tc.tile_pool(name="pmm", bufs=2, space="PSUM"))
        gdp = ctx.enter_context(tc.tile_pool(name="gdp", bufs=1, space="DRAM"))

        ident = consts.tile([128, 128], fp32)
        make_identity(nc, ident)
        ones_row = consts.tile([1, 128], fp32)
        nc.vector.memset(ones_row, 1.0)
        ones_col = consts.tile([128, 1], fp32)
        nc.vector.memset(ones_col, 1.0)

        S = [dict() for _ in STREAMS]  # cross-phase per-stream state

        def p1setup(si):
            p, N, K = STREAMS[si]
            KT = (K + 127) // 128

            # weights (SP: no dependencies at issue)
            W1t = swp.tile([128, 2, D], fp32, tag="W1t")
            nc.sync.dma_start(
                out=W1t, in_=wts[(p, "W1")][:].rearrange("(a k) m -> k a m", k=128)
            )
            W2t = swp.tile([128, 2, N * D], fp32, tag="W2t")
            nc.sync.dma_start(
                out=W2t, in_=wts[(p, "W2")][:].rearrange("(a k) m -> k a m", k=128)
            )
            dW2t = swp.tile([128, 2, D], fp32, tag="dW2t")
            nc.sync.dma_start(
                out=dW2t, in_=wts[(p, "dW2")][:].rearrange("(a k) m -> k a m", k=128)
            )
            b1c = swp.tile([128, 2], fp32, tag="b1c")
            nc.sync.dma_start(
                out=b1c, in_=wts[(p, "b1")][:].rearrange("(m k) -> k m", k=128)
            )
            b2c = swp.tile([128, 2 * N], fp32, tag="b2c")
            nc.sync.dma_start(
                out=b2c, in_=wts[(p, "b2")][:].rearrange("(m k) -> k m", k=128)
            )
            db1f2 = swp.tile([128, D], fp32, tag="db1f2")
            nc.sync.dma_start(
                out=db1f2,
                in_=wts[(p, "db1")][:]
                .rearrange("(a d) -> a d", a=1)
                .to_broadcast([128, D]),
            )
            nc.scalar.activation(out=db1f2, in_=db1f2, func=Act.Copy, scale=2.0)
            db2f = swp.tile([128, D], fp32, tag="db2f")
            nc.sync.dma_start(
                out=db2f,
                in_=wts[(p, "db2")][:]
                .rearrange("(a d) -> a d", a=1)
                .to_broadcast([128, D]),
            )

            # prefetch all x chunks up front
            x4s = []
            for c in range(NCH):
                x4 = chk.tile([128, RT, D], fp32, tag="x4", bufs=NCH)
                nc.sync.dma_start(
                    out=x4,
                    in_=x_in[p][c * CH : (c + 1) * CH, :].rearrange(
                        "(j q) d -> q j d", q=128
                    ),
                )
                x4s.append(x4)

            # transpose codebook: cbT2[:, h*K + j] = 2 * cb[j, h*128 + p]
            cbT2 = swp.tile([128, 2 * K], fp32, tag="cbT2")
            for kt in range(KT):
                ksz = min(128, K - kt * 128)
                cbrow = swp.tile([128, D], fp32, tag="cbrow", bufs=3)
                nc.sync.dma_start(
                    out=cbrow[:ksz], in_=wts[(p, "cb")][kt * 128 : kt * 128 + ksz, :]
                )
                for h in range(2):
                    tps = ptr.tile([128, 128], fp32, tag="tr")
                    nc.tensor.transpose(
                        out=tps[:, :ksz],
                        in_=cbrow[:ksz, h * 128 : (h + 1) * 128],
                        identity=ident[:ksz, :ksz],
                    )
                    nc.scalar.activation(
                        out=cbT2[:, h * K + kt * 128 : h * K + kt * 128 + ksz],
                        in_=tps[:, :ksz],
                        func=Act.Copy,
                        scale=2.0,
                    )

            # c2 = sum_d cb[k,d]^2, materialized across partitions
            sqT = swp.tile([128, 2 * K], fp32, tag="sqT")
            nc.vector.tensor_tensor(out=sqT, in0=cbT2, in1=cbT2, op=Alu.mult)
            c2s = swp.tile([1, K], fp32, tag="c2s")
            for cb_ofs in range(0, K, 512):
                csz = min(512, K - cb_ofs)
                pc2 = pmm.tile([1, 512], fp32, tag="dist", bufs=2)
                for h in range(2):
                    nc.tensor.matmul(
                        out=pc2[:, :csz],
                        lhsT=ones_col,
                        rhs=sqT[:, h * K + cb_ofs : h * K + cb_ofs + csz],
                        start=(h == 0),
                        stop=(h == 1),
                    )
                nc.scalar.activation(
                    out=c2s[:, cb_ofs : cb_ofs + csz],
                    in_=pc2[:, :csz],
                    func=Act.Copy,
                    scale=0.25,
                )
            c2d = gdp.tile([1, K], fp32, tag=f"c2d_{p}", name=f"c2d_{p}")
            nc.scalar.dma_start(out=c2d, in_=c2s)
            c2full = swp.tile([128, K], fp32, tag="c2full")
            nc.scalar.dma_start(out=c2full, in_=c2d[:].to_broadcast([128, K]))

            # G'_n = 2*cb @ dW1_n (+ 2*db1 folded into G'_0), concatenated
            # into one (N*K, D) table so one dma_gather serves all N codes
            gcat = gdp.tile([N * K, D], fp32, tag=f"gcat_{p}", name=f"gcat_{p}")
            stg = idx_d[p]
            offc = swp.tile([128, N], i16, tag="offc")
            for n in range(N):
                nc.vector.memset(offc[:, n : n + 1], n * K)
            for n in range(N):
                dW1n = gwk.tile([128, 2, D], fp32, tag="dW1n")
                nc.sync.dma_start(
                    out=dW1n,
                    in_=wts[(p, "dW1")][n * D : (n + 1) * D, :].rearrange(
                        "(a k) m -> k a m", k=128
                    ),
                )
                for mt in range(KT):
                    msz = min(128, K - mt * 128)
                    pg = pmm.tile([128, D], fp32, tag="dist", bufs=2)
                    for h in range(2):
                        nc.tensor.matmul(
                            out=pg[:msz],
                            lhsT=cbT2[:, h * K + mt * 128 : h * K + mt * 128 + msz],
                            rhs=dW1n[:, h, :],
                            start=(h == 0),
                            stop=(h == 1),
                        )
                    gsb = gwk.tile([128, D], fp32, tag="gsb", bufs=3)
                    if n == 0:
                        # every row of table 0 also gets +2*db1
                        nc.vector.tensor_tensor(
                            out=gsb[:msz], in0=pg[:msz], in1=db1f2[:msz],
                            op=Alu.add,
                        )
                    else:
                        nc.scalar.activation(
                            out=gsb[:msz], in_=pg[:msz], func=Act.Copy
                        )
                    nc.scalar.dma_start(
                        out=gcat[n * K + mt * 128 : n * K + mt * 128 + msz, :],
                        in_=gsb[:msz],
                    )

            # per-stream accumulators
            macc = acc.tile([128, 1], fp32, tag="macc")
            nc.vector.memset(macc, 0.0)
            zacc = acc.tile([128, 1], fp32, tag="zacc")
            nc.vector.memset(zacc, 0.0)

            S[si].update(
                W1t=W1t, W2t=W2t, dW2t=dW2t, b1c=b1c, b2c=b2c, db2f=db2f,
                cbT2=cbT2, c2full=c2full, gcat=gcat, stg=stg, offc=offc,
                macc=macc, zacc=zacc, x4s=x4s,
            )

        def p1chunk(si, c):
            p, N, K = STREAMS[si]
            KB = (K + 511) // 512
            st = S[si]
            x4 = st["x4s"][c]

            xT = chk.tile([128, 2 * CH], fp32, tag="xT")
            for j in range(RT):
                for h in range(2):
                    tps = ptr.tile([128, 128], fp32, tag="tr")
                    nc.tensor.transpose(
                        out=tps, in_=x4[:, j, h * 128 : (h + 1) * 128],
                        identity=ident,
                    )
                    nc.scalar.activation(
                        out=xT[:, h * CH + j * 128 : h * CH + (j + 1) * 128],
                        in_=tps,
                        func=Act.Copy,
                    )

            # encoder layer 1: h1T = relu(W1-slices^T @ xT + b1)
            h1T = chk.tile([128, 2 * CH], fp32, tag="h1T")
            for m in range(2):
                ph = pmm.tile([128, CH], fp32, tag="enc", bufs=2)
                for h in range(2):
                    nc.tensor.matmul(
                        out=ph,
                        lhsT=st["W1t"][:, h, m * 128 : (m + 1) * 128],
                        rhs=xT[:, h * CH : (h + 1) * CH],
                        start=(h == 0),
                        stop=(h == 1),
                    )
                nc.scalar.activation(
                    out=h1T[:, m * CH : (m + 1) * CH],
                    in_=ph,
                    func=Act.Relu,
                    bias=st["b1c"][:, m : m + 1],
                )

            # encoder layer 2: zT = W2-slices^T @ h1T + b2
            zT = chk.tile([128, 2 * N * CH], fp32, tag="zT")
            for m in range(2 * N):
                pz = pmm.tile([128, CH], fp32, tag="enc", bufs=2)
                for h in range(2):
                    nc.tensor.matmul(
                        out=pz,
                        lhsT=st["W2t"][:, h, m * 128 : (m + 1) * 128],
                        rhs=h1T[:, h * CH : (h + 1) * CH],
                        start=(h == 0),
                        stop=(h == 1),
                    )
                nc.scalar.activation(
                    out=zT[:, m * CH : (m + 1) * CH],
                    in_=pz,
                    func=Act.Identity,
                    bias=st["b2c"][:, m : m + 1],
                )
                # |z|^2 partial sums (for the loss)
                zsq = acc.tile([128, CH], fp32, tag="zsq")
                z2t = acc.tile([128, 1], fp32, tag="z2t", bufs=4)
                nc.scalar.activation(
                    out=zsq,
                    in_=zT[:, m * CH : (m + 1) * CH],
                    func=Act.Square,
                    accum_out=z2t,
                )
                nc.vector.tensor_tensor(
                    out=st["zacc"], in0=st["zacc"], in1=z2t, op=Alu.add
                )

            # VQ per 128-row tile
            last_mm = None
            last_store = None
            for r in range(RT):
                rt = c * RT + r
                idx16c = vqp.tile([128, N], i16, tag="idx16c")
                for n in range(N):
                    nd = vqp.tile([128, K], fp32, tag="nd")
                    maxv = vqp.tile([128, 8], fp32, tag="maxv", bufs=4)
                    nc.vector.memset(maxv, NEG)
                    for b in range(KB):
                        ksz = min(512, K - b * 512)
                        pd = pmm.tile([128, 512], fp32, tag="dist", bufs=2)
                        for h in range(2):
                            last_mm = nc.tensor.matmul(
                                out=pd[:, :ksz],
                                lhsT=zT[
                                    :,
                                    (2 * n + h) * CH
                                    + r * 128 : (2 * n + h) * CH
                                    + (r + 1) * 128,
                                ],
                                rhs=st["cbT2"][
                                    :, h * K + b * 512 : h * K + b * 512 + ksz
                                ],
                                start=(h == 0),
                                stop=(h == 1),
                            )
                        # nd = pd - c2   (== |z|^2 - d, up to a row const)
                        nc.vector.tensor_tensor(
                            out=nd[:, b * 512 : b * 512 + ksz],
                            in0=pd[:, :ksz],
                            in1=st["c2full"][:, b * 512 : b * 512 + ksz],
                            op=Alu.subtract,
                        )
                    nc.vector.tensor_reduce(
                        out=maxv[:, 0:1], in_=nd, axis=mybir.AxisListType.X,
                        op=Alu.max,
                    )
                    idx8 = vqp.tile([128, 8], u16, tag="idx8", bufs=4)
                    nc.vector.max_index(out=idx8, in_max=maxv, in_values=nd)
                    # loss: macc -= max  (sum of min-dist partials)
                    nc.vector.tensor_tensor(
                        out=st["macc"], in0=st["macc"], in1=maxv[:, 0:1],
                        op=Alu.subtract,
                    )
                    # idx + n*K (int16): gather offsets AND index output
                    nc.vector.tensor_tensor(
                        out=idx16c[:, n : n + 1],
                        in0=idx8[:, 0:1].bitcast(i16),
                        in1=st["offc"][:, n : n + 1],
                        op=Alu.add,
                    )

                # stage idx16 to DRAM; chunk layout: m = r*(N*128)+n*128+p
                # (GpSimd: waits on the VQ tail, nothing urgent behind it)
                last_store = nc.gpsimd.dma_start(
                    out=bass.AP(
                        tensor=st["stg"],
                        offset=(c * RT + r) * N * 128,
                        ap=[[1, 128], [128, N]],
                    ),
                    in_=idx16c,
                )

            st.setdefault("anchor_mm", {})[c] = last_mm
            st.setdefault("anchor_store", {})[c] = last_store

        def p1epilogue(si):
            p, N, K = STREAMS[si]
            st = S[si]
            nc.gpsimd.dma_start(
                out=lossm_d[si, :].rearrange("(a q) -> q a", a=1), in_=st["macc"]
            )
            nc.gpsimd.dma_start(
                out=lossz_d[si, :].rearrange("(a q) -> q a", a=1), in_=st["zacc"]
            )

        def p2chunk(si, c, anchor_mm=None, anchor_store=None):
            p, N, K = STREAMS[si]
            st = S[si]
            # reload idx16 in the 16-partition wrap layout, replicated across
            # the 8 partition groups (each GpSimd core reads its own):
            # idxs[g*16+q, r, n, j] = stg[c, r*N*128 + n*128 + j*16 + q]
            idx16w = dec.tile([128, RT, N, 8], i16, tag="idx16w", bufs=2)
            for g in range(8):
                nc.sync.dma_start(
                    out=idx16w[g * 16 : (g + 1) * 16, :, :, :],
                    in_=bass.AP(
                        tensor=st["stg"],
                        offset=c * (RT * N * 128),
                        ap=[[1, 16], [N * 128, RT], [128, N], [16, 8]],
                    ),
                )

            for r in range(RT):
                rt = c * RT + r
                wide = dec.tile([128, N, D], fp32, tag="wide")
                gthr = nc.gpsimd.dma_gather(
                    out_ap=wide,
                    in_ap=st["gcat"][:],
                    idxs_ap=idx16w[:, r, :, :].rearrange("q n j -> q (n j)"),
                    num_idxs=N * 128,
                    num_idxs_reg=N * 128,
                    elem_size=D,
                )
                if anchor_store is not None:
                    add_dep_helper(
                        gthr.ins, anchor_store.ins, sync=False,
                        reason="lag decode gathers behind the pipelined VQ",
                    )
                pre = dec.tile([128, D], fp32, tag="pre")
                nc.vector.tensor_tensor(
                    out=pre, in0=wide[:, 0, :], in1=wide[:, 1, :], op=Alu.add
                )
                for n in range(2, N):
                    nc.vector.tensor_tensor(
                        out=pre, in0=pre, in1=wide[:, n, :], op=Alu.add
                    )

                # decoder: h1d = relu(0.5*pre) ; out = h1dT-mm dW2 + db2
                h1d = dec.tile([128, D], fp32, tag="h1d")
                nc.scalar.activation(out=h1d, in_=pre, func=Act.Relu, scale=0.5)
                h1dT = dec.tile([128, 2 * 128], fp32, tag="h1dT")
                for h in range(2):
                    tps = ptr.tile([128, 128], fp32, tag="tr")
                    ti = nc.tensor.transpose(
                        out=tps, in_=h1d[:, h * 128 : (h + 1) * 128], identity=ident
                    )
                    if anchor_mm is not None:
                        add_dep_helper(
                            ti.ins, anchor_mm.ins, sync=False,
                            reason="keep decode PE work behind this chunk's VQ",
                        )
                    nc.scalar.activation(
                        out=h1dT[:, h * 128 : (h + 1) * 128], in_=tps, func=Act.Copy
                    )
                po = pmm.tile([128, D], fp32, tag="po", bufs=2)
                for h in range(2):
                    nc.tensor.matmul(
                        out=po,
                        lhsT=h1dT[:, h * 128 : (h + 1) * 128],
                        rhs=st["dW2t"][:, h, :],
                        start=(h == 0),
                        stop=(h == 1),
                    )
                outt = dec.tile([128, D], fp32, tag="outt")
                nc.vector.tensor_tensor(
                    out=outt, in0=po, in1=st["db2f"], op=Alu.add
                )
                nc.scalar.dma_start(
                    out=out_d[rt * 128 : (rt + 1) * 128, si, :], in_=outt
                )

        # ---- software pipeline: p2 runs two chunks behind the next
        # stream's p1 ----
        LAG = 2
        p1setup(0)
        for c in range(NCH):
            p1chunk(0, c)
        p1epilogue(0)
        p1setup(1)
        for c in range(NCH):
            p1chunk(1, c)
            if c >= LAG:
                p2chunk(
                    0, c - LAG,
                    anchor_mm=S[1]["anchor_mm"][c],
                    anchor_store=S[1]["anchor_store"][c - 1],
                )
        p1epilogue(1)
        for c in range(max(0, NCH - LAG), NCH):
            p2chunk(0, c)
        p1setup(2)
        for c in range(NCH):
            p1chunk(2, c)
            if c >= LAG:
                p2chunk(
                    1, c - LAG,
                    anchor_mm=S[2]["anchor_mm"][c],
                    anchor_store=S[2]["anchor_store"][c - 1],
                )
                p2chunk(
                    2, c - LAG,
                    anchor_mm=S[2]["anchor_mm"][c],
                    anchor_store=S[2]["anchor_store"][c - 1],
                )
        p1epilogue(2)
        for c in range(max(0, NCH - LAG), NCH):
            p2chunk(1, c)
        for c in range(max(0, NCH - LAG), NCH):
            p2chunk(2, c)

    nc.compile()
    return nc


def _get_nc(rows=BLOC, debug=False):
    key = (rows, debug)
    if key not in _BUILT:
        _BUILT[key] = build(rows, debug=debug)
    return _BUILT[key]


def make_in_map(inputs, core, rows=BLOC):
    sl = slice(core * rows, (core + 1) * rows)
    m = {
        "x_t": np.ascontiguousarray(np.asarray(inputs["sketch_features"])[sl]),
        "x_g": np.ascontiguousarray(np.asarray(inputs["geometry_features"])[sl]),
        "x_e": np.ascontiguousarray(np.asarray(inputs["extrusion_features"])[sl]),
    }
    for p, N, K in STREAMS:
        for w in ("W1", "b1", "W2", "b2", "cb", "dW1", "db1", "dW2", "db2"):
            m[f"{p}_{w}"] = np.ascontiguousarray(
                np.asarray(inputs[f"{p}_{w}"], dtype=np.float32)
            )
    return m


def assemble(results):
    """results: list of 8 per-core output dicts -> reference-shaped outputs."""
    out = np.concatenate([r["out"] for r in results], axis=0)
    idxs = {}
    for p, N, K in STREAMS:
        percore = []
        for r in results:
            a = r[f"idxs_{p}"].astype(np.int32)  # (NCH, RT*N*128), idx + n*K
            nch = a.shape[0]
            a = a.reshape(nch, -1, N, 128) - (np.arange(N, dtype=np.int32) * K)[
                None, None, :, None
            ]
            # value[c, r, n, q] is code for row c*CH + r*128 + q, slot n
            percore.append(np.ascontiguousarray(a.transpose(0, 1, 3, 2)).reshape(-1))
        idxs[p] = np.concatenate(percore, axis=0)
    loss = np.float32(0.0)
    for si, (p, N, K) in enumerate(STREAMS):
        tot = np.float32(0.0)
        for r in results:
            tot += r["loss_m"][si].sum(dtype=np.float32) + r["loss_z"][si].sum(
                dtype=np.float32
            )
        loss = np.float32(loss + np.float32(BETA) * tot / np.float32(B * N * D))
    return out, loss, idxs["t"], idxs["g"], idxs["e"]


def kernel(**inputs):
    from concourse import bass_utils

    nc = _get_nc()
    in_maps = [make_in_map(inputs, c) for c in range(NCORES)]
    res = bass_utils.run_bass_kernel_spmd(nc, in_maps, core_ids=list(range(NCORES)))
    return assemble(res.results)
